# revision 18
# baseline (speedup 1.0000x reference)
"""Causal self-attention with RoPE on 8 trn2 NeuronCores (Bass/Tile).

Sharding: tensor-parallel over heads (4 heads/core) x data-parallel over
batch (B=2). Core i = b*4 + t handles batch b, heads 4t..4t+3.

Per-core dataflow (all matmuls bf16, fp32 PSUM):
  - host passes x.T [C, T] so contractions always have K on partitions
  - qk.T [512(j), T] = w_qk.T @ x.T   (lhsT = w_qk natural [c, j])
  - RoPE on q.T/k.T in [d, t] layout; per-head d-columns are permuted to
    [evens | odds] so the rotation is 32-partition-block aligned; the
    32-block swap is 2 batched SBUF-SBUF DMAs issued on the scalar /
    vector queues (keeps the sync queue free for input/output streaming)
  - v [T, 256] natural (lhsT = x.T tile) + ones column (aug) per head
  - S.T [k, q] per head = (k.T).T @ q.T   (K = d = 64; heads of a pair
    use disjoint PE row groups -> partial dual-issue)
  - causal trim: for the 4 diagonal k-tiles of each q-chunk only columns
    [off, TC) are computed (scores, exp, AV); the mixed 128-col band gets
    a single [128,128] tril mask multiply (DVE 4x mode); fully-masked
    regions are never touched
  - P = exp(0.125 * S.T) on ScalarE (no max subtraction; |s| is small)
  - y_aug.T [65, q] = v_aug.T @ P  accumulated over k tiles; row 64 is
    the softmax denominator (ones column)
  - y psum is evacuated to SBUF bf16 via ScalarE Copy as soon as the AV
    accumulation closes (frees the PSUM bank; the reciprocal+broadcast
    normalize chain then runs off the critical path on gpsimd DMAs)
  - out_partial.T [C, q] = w_proj_shard.T @ y.T ; host sums the 4
    partials of each batch and transposes.

Program order interleaves the QKV segments and the projection of older
chunks with the attention chunks via a persistent filler queue so the
TensorEngine stays dense while ScalarE works through the exp volume.
"""

import numpy as np
import ml_dtypes

B, T, C, H = 2, 2048, 1024, 16
HD = C // H          # 64
HPC = H // 4         # heads per core = 4
JQK = 2 * HPC * HD   # 512  (q|k columns per core)
JV = HPC * HD        # 256
N_CORES = 8
TC = 512             # q/t chunk (moving free dim)
NQC = T // TC        # 4 q-chunks
NKT = T // 128       # 16 k-tiles
NCT = C // 128       # 8 contraction tiles
VGW = 66             # v group width per head: 64 v cols + 1 ones + 1 pad
ST_G = 2             # score psum group (k-tiles per exp)

_CACHE = {}


def _build():
    import concourse.bass as bass
    import concourse.tile as tile
    from concourse import bacc, mybir
    EXP = mybir.ActivationFunctionType.Exp
    COPY = mybir.ActivationFunctionType.Copy

    bf16 = mybir.dt.bfloat16
    f32 = mybir.dt.float32

    nc = bacc.Bacc("TRN2", target_bir_lowering=False, debug=False,
                   num_devices=N_CORES)
    xT = nc.dram_tensor("xT", [C, T], bf16, kind="ExternalInput").ap()
    w_qk = nc.dram_tensor("w_qk", [C, JQK], bf16, kind="ExternalInput").ap()
    w_v = nc.dram_tensor("w_v", [C, JV], bf16, kind="ExternalInput").ap()
    w_pr = nc.dram_tensor("w_pr", [JV, C], bf16, kind="ExternalInput").ap()
    cos_p = nc.dram_tensor("cos_p", [128, T], bf16, kind="ExternalInput").ap()
    sin_p = nc.dram_tensor("sin_p", [128, T], bf16, kind="ExternalInput").ap()
    trimask = nc.dram_tensor("trimask", [128, 128], bf16,
                             kind="ExternalInput").ap()
    yT_out = nc.dram_tensor("yT", [C, T], bf16, kind="ExternalOutput").ap()

    with tile.TileContext(nc) as tc:
        import contextlib
        ctx = contextlib.ExitStack()
        with ctx:
            const = ctx.enter_context(tc.tile_pool(name="const", bufs=1))
            ppool = ctx.enter_context(tc.tile_pool(name="p", bufs=3))
            ypool = ctx.enter_context(tc.tile_pool(name="ysb", bufs=4))
            rpool = ctx.enter_context(tc.tile_pool(name="r", bufs=4))
            ohpool = ctx.enter_context(tc.tile_pool(name="oh", bufs=8))
            # PSUM: shared pool 3 slots x 2 banks + 2 y accumulators x 1
            # bank = 8 banks
            mm_ps = ctx.enter_context(
                tc.tile_pool(name="mmps", bufs=3, space="PSUM"))
            y_ps = ctx.enter_context(
                tc.tile_pool(name="yps", bufs=2, space="PSUM"))
            dram = ctx.enter_context(
                tc.tile_pool(name="dram", bufs=4, space="DRAM"))

            # ---- resident inputs.  Chunked/interleaved so segment 0's
            # accumulation can start as soon as its (w, x) c-tiles land;
            # spread across queues so the sync queue only carries what
            # gates early compute. ----
            t_xT = const.tile([128, NCT, T], bf16)
            t_wqk = const.tile([128, NCT, JQK], bf16)
            t_wv = const.tile([128, NCT, JV], bf16)
            t_cos = const.tile([128, T], bf16)
            t_sin = const.tile([128, T], bf16)
            t_mask = const.tile([128, 128], bf16)
            t_wpr = const.tile([128, 2, C], bf16)
            # two issue queues in parallel; first-needed first.  sync:
            # x chunk 0 then w_v then x rest; scalar: w_qk (+cos/sin
            # after the first two c-tiles, they gate the first RoPE).
            for ci in range(2):
                nc.scalar.dma_start(out=t_wqk[:, ci, :],
                                    in_=w_qk[ci * 128:(ci + 1) * 128, :])
                nc.sync.dma_start(out=t_xT[:, ci, 0:TC],
                                  in_=xT[ci * 128:(ci + 1) * 128, 0:TC])
            nc.scalar.dma_start(out=t_cos, in_=cos_p)
            nc.scalar.dma_start(out=t_sin, in_=sin_p)
            for ci in range(2, NCT):
                nc.scalar.dma_start(out=t_wqk[:, ci, :],
                                    in_=w_qk[ci * 128:(ci + 1) * 128, :])
                nc.sync.dma_start(out=t_xT[:, ci, 0:TC],
                                  in_=xT[ci * 128:(ci + 1) * 128, 0:TC])
            for ci in range(NCT):
                nc.sync.dma_start(out=t_wv[:, ci, :],
                                  in_=w_v[ci * 128:(ci + 1) * 128, :])
            nc.scalar.dma_start(out=t_mask, in_=trimask)
            for ci in range(2):
                nc.scalar.dma_start(out=t_wpr[:, ci, :],
                                    in_=w_pr[ci * 128:(ci + 1) * 128, :])
            for ci in range(NCT):
                nc.sync.dma_start(out=t_xT[:, ci, TC:T],
                                  in_=xT[ci * 128:(ci + 1) * 128, TC:T])

            # qk.T buffer: [128, jt, T]; jt 0..1 = q head-pairs, 2..3 = k
            t_qkT = const.tile([128, 4, T], bf16)

            # v buffer: [128(t), kt_hi, 4*66]; per head 64 v + ones + pad
            t_v = const.tile([128, NKT, 4 * VGW], bf16)
            vv = t_v.rearrange("p k (h c) -> p k h c", h=4)
            for h in range(4):
                nc.vector.memset(vv[:, :, h, 64:65], 1.0)
            # fp32 ones rows (partition 0 and 32, matching the recip
            # rows) for the tail PE-broadcast of 1/denominator
            t_one = const.tile([33, 64], f32)
            nc.vector.memset(t_one[0:1], 1.0)
            nc.vector.memset(t_one[32:33], 1.0)

            swap_q = [nc.scalar, nc.gpsimd]

            def segment_blocks(tcn):
                """yield per-block callables: 4 qk(+rope) blocks then 4
                v blocks for t-chunk tcn."""
                sl = slice(tcn * TC, (tcn + 1) * TC)

                def qk_block(jt, sl=sl):
                    ps = mm_ps.tile([128, TC], f32, tag="mm", name="psqk")
                    for ci in range(NCT):
                        nc.tensor.matmul(
                            ps,
                            lhsT=t_wqk[:, ci, jt * 128:(jt + 1) * 128],
                            rhs=t_xT[:, ci, sl],
                            start=(ci == 0), stop=(ci == NCT - 1))
                    q = t_qkT[:, jt, sl]
                    # RoPE (even/odd-split): o_e = e*cos - o*sin,
                    # o_o = o*cos + e*sin; sinP rows [+s, -s, +s, -s].
                    # 32-block swap goes through DMA (DVE needs equal
                    # base partitions for SBUF+SBUF inputs); 2 batched
                    # DMAs on the scalar/vector queues.
                    u = rpool.tile([128, TC], bf16, tag="ru")
                    w = rpool.tile([128, TC], bf16, tag="rw")
                    ws = rpool.tile([128, TC], bf16, tag="rws")
                    nc.vector.tensor_mul(out=u, in0=ps, in1=t_cos[:, sl])
                    nc.vector.tensor_mul(out=w, in0=ps, in1=t_sin[:, sl])
                    eng = swap_q[jt % 2]
                    for b0 in range(0, 128, 32):
                        eng.dma_start(out=ws[b0:b0 + 32, :],
                                      in_=w[b0 ^ 32:(b0 ^ 32) + 32, :])
                    nc.vector.tensor_add(out=q, in0=u, in1=ws)

                def v_block(tt):
                    ps = mm_ps.tile([128, JV], f32, tag="mm", name="psv")
                    for ci in range(NCT):
                        nc.tensor.matmul(
                            ps,
                            lhsT=t_xT[:, ci, tt * 128:(tt + 1) * 128],
                            rhs=t_wv[:, ci, :],
                            start=(ci == 0), stop=(ci == NCT - 1))
                    nc.vector.tensor_copy(
                        out=vv[:, tt, :, 0:64],
                        in_=ps.rearrange("p (h c) -> p h c", h=4))

                for jt in range(4):
                    yield (lambda j=jt: qk_block(j))
                for tt in range(4 * tcn, 4 * tcn + 4):
                    yield (lambda t=tt: v_block(t))

            def attn_half(qc, hp, y_qc, filler=None, last=False):
                """scores+softmax+AV+normalize for q-chunk qc, head pair
                hp.  Causally trimmed: diagonal k-tile kt (off = kt*128
                - qc*TC in [0, TC)) only computes columns [off, TC).
                AV for group g is emitted after ST of group g+2 (its exp
                has landed by then); `filler` supplies extra PE work."""
                nkt = 4 * (qc + 1)
                qsl = slice(qc * TC, (qc + 1) * TC)
                pA = ppool.tile([128, NKT, TC], bf16, tag="pbuf")
                pB = ppool.tile([128, NKT, TC], bf16, tag="pbuf")
                yA = y_ps.tile([65, TC], f32, tag="yps")
                yB = y_ps.tile([65, TC], f32, tag="yps")

                def off_of(kt):
                    off = kt * 128 - qc * TC
                    return off if 0 <= off < TC else 0

                def av(kt):
                    off = off_of(kt)
                    for half, (yps, p) in enumerate(((yA, pA), (yB, pB))):
                        h = 2 * hp + half
                        nc.tensor.matmul(
                            yps[:, off:],
                            lhsT=t_v[:, kt, h * VGW:h * VGW + 65],
                            rhs=p[:, kt, off:],
                            start=(kt == 0), stop=(kt == nkt - 1))

                ngrp = (nkt + ST_G - 1) // ST_G
                for g in range(ngrp):
                    g0 = g * ST_G
                    gl = min(ST_G, nkt - g0)
                    stA = mm_ps.tile([128, ST_G, TC], f32, tag="mm")
                    stB = mm_ps.tile([128, ST_G, TC], f32, tag="mm")
                    offs = []
                    for kg in range(gl):
                        kt = g0 + kg
                        off = off_of(kt)
                        offs.append(off)
                        ksl = slice(kt * 128, (kt + 1) * 128)
                        qssl = slice(qc * TC + off, (qc + 1) * TC)
                        nc.tensor.matmul(
                            stA[:, kg, off:],
                            lhsT=t_qkT[0:64, 2 + hp, ksl],
                            rhs=t_qkT[0:64, hp, qssl],
                            start=True, stop=True)
                        nc.tensor.matmul(
                            stB[:, kg, off:],
                            lhsT=t_qkT[64:128, 2 + hp, ksl],
                            rhs=t_qkT[64:128, hp, qssl],
                            start=True, stop=True)
                    # AV for the group 2 back, plus periodic filler to
                    # cover the PE-vs-ACT deficit
                    if g >= 2:
                        for kt in range((g - 2) * ST_G,
                                        (g - 2) * ST_G + ST_G):
                            av(kt)
                    if filler is not None and (g < 2 or g % 3 == 2):
                        f = next(filler, None)
                        if f is not None:
                            f()
                    if offs[0] == 0 and (gl == 1 or offs[1] == 0):
                        # full-width group: one ACT per half
                        nc.scalar.activation(
                            out=pA[:, g0:g0 + gl, :], in_=stA[:, 0:gl, :],
                            func=EXP, scale=0.125)
                        nc.scalar.activation(
                            out=pB[:, g0:g0 + gl, :], in_=stB[:, 0:gl, :],
                            func=EXP, scale=0.125)
                    else:
                        for kg in range(gl):
                            kt = g0 + kg
                            off = offs[kg]
                            nc.scalar.activation(
                                out=pA[:, kt, off:], in_=stA[:, kg, off:],
                                func=EXP, scale=0.125)
                            nc.scalar.activation(
                                out=pB[:, kt, off:], in_=stB[:, kg, off:],
                                func=EXP, scale=0.125)
                    for kg in range(gl):
                        kt = g0 + kg
                        off = kt * 128 - qc * TC
                        if 0 <= off < TC:  # diagonal tile -> band mask
                            nc.vector.tensor_mul(
                                out=pA[:, kt, off:off + 128],
                                in0=pA[:, kt, off:off + 128], in1=t_mask)
                            nc.vector.tensor_mul(
                                out=pB[:, kt, off:off + 128],
                                in0=pB[:, kt, off:off + 128], in1=t_mask)
                for kt in range(max(0, (ngrp - 2) * ST_G), nkt):
                    av(kt)
                # evacuate y+denominator rows to SBUF (frees the psum banks);
                # reciprocal + DRAM broadcast runs on gpsimd off the
                # critical path, final scale on DVE in fast bf16 mode.
                y2a = rpool.tile([65, TC], bf16, tag="y2a")
                y2b = rpool.tile([65, TC], bf16, tag="y2b")
                nc.scalar.activation(out=y2a, in_=yA, func=COPY)
                nc.scalar.activation(out=y2b, in_=yB, func=COPY)
                d2 = rpool.tile([64, TC], f32, tag="d2")
                r2 = rpool.tile([64, TC], f32, tag="r2")
                nc.vector.tensor_copy(out=d2[0:1, :], in_=y2a[64:65, :])
                nc.vector.tensor_copy(out=d2[32:33, :], in_=y2b[64:65, :])
                nc.vector.reciprocal_approx_fast(out=r2, in_=d2)
                if last:
                    # tail: PE is idle — broadcast 1/den via fp32 matmul
                    # (ones[1,64].T @ r2_row) instead of the DRAM bounce
                    rpsA = y_ps.tile([64, TC], f32, tag="yps")
                    rpsB = y_ps.tile([64, TC], f32, tag="yps")
                    nc.tensor.matmul(rpsA, lhsT=t_one[0:1], rhs=r2[0:1, :],
                                     start=True, stop=True)
                    nc.tensor.matmul(rpsB, lhsT=t_one[32:33],
                                     rhs=r2[32:33, :],
                                     start=True, stop=True)
                    nc.vector.tensor_mul(
                        out=y_qc[0:64, hp, :], in0=y2a[0:64, :], in1=rpsA)
                    nc.vector.tensor_mul(
                        out=y_qc[64:128, hp, :], in0=y2b[0:64, :], in1=rpsB)
                else:
                    rd = dram.tile([2, TC], f32, tag="rd")
                    nc.sync.dma_start(
                        out=rd,
                        in_=r2.rearrange("(a b) t -> a b t", b=32)[:, 0, :])
                    r64a = rpool.tile([64, TC], bf16, tag="r64a")
                    r64b = rpool.tile([64, TC], bf16, tag="r64b")
                    nc.gpsimd.dma_start(
                        out=r64a, in_=rd[0:1, :].to_broadcast((64, TC)))
                    nc.gpsimd.dma_start(
                        out=r64b, in_=rd[1:2, :].to_broadcast((64, TC)))
                    nc.vector.tensor_mul(
                        out=y_qc[0:64, hp, :], in0=y2a[0:64, :], in1=r64a)
                    nc.vector.tensor_mul(
                        out=y_qc[64:128, hp, :], in0=y2b[0:64, :], in1=r64b)

            def proj_blocks(qc, y_qc):
                for co in range(NCT):
                    def co_block(co=co):
                        ps = mm_ps.tile([128, TC], f32, tag="mm", name="psp")
                        for ci in range(2):
                            nc.tensor.matmul(
                                ps,
                                lhsT=t_wpr[:, ci, co * 128:(co + 1) * 128],
                                rhs=y_qc[:, ci, :],
                                start=(ci == 0), stop=(ci == 1))
                        o_sb = rpool.tile([128, TC], bf16, tag="osb")
                        nc.vector.tensor_copy(out=o_sb, in_=ps)
                        nc.sync.dma_start(
                            out=yT_out[co * 128:(co + 1) * 128,
                                       qc * TC:(qc + 1) * TC],
                            in_=o_sb)
                    yield co_block

            # interleave segments with attention chunks: while ScalarE
            # works through a chunk's exp volume, PE has segment/proj
            # matmuls.  A persistent filler queue lets surplus work from
            # early (ACT-light) chunks spill into late (ACT-heavy) ones;
            # segment blocks are force-drained before their consumer.
            from collections import deque
            y_qcs = [None] * NQC
            segment_due = [deque(segment_blocks(tcn)) for tcn in range(NQC)]
            proj_due = deque()

            def drain(dq):
                while dq:
                    dq.popleft()()

            def filler_iter():
                while True:
                    for tcn in range(NQC):
                        if segment_due[tcn]:
                            yield segment_due[tcn].popleft()
                            break
                    else:
                        if proj_due:
                            yield proj_due.popleft()
                        else:
                            yield None

            fill = filler_iter()
            drain(segment_due[0])
            drain(segment_due[1])
            o_half = [None] * NCT
            for qc in range(NQC):
                # correctness: chunk qc's q/k/v must exist before use
                drain(segment_due[qc])
                y_qc = ypool.tile([128, 2, TC], bf16, tag="yqc")
                y_qcs[qc] = y_qc
                if qc >= 1:
                    proj_due.extend(proj_blocks(qc - 1, y_qcs[qc - 1]))
                attn_half(qc, 0, y_qc, fill)
                if qc == NQC - 1:
                    # tail shrink: the hp=0 half of the last chunk's
                    # projection can run while hp=1 attention finishes
                    def pass1(co, y_qc=y_qc):
                        ps = mm_ps.tile([128, TC], f32, tag="mm")
                        nc.tensor.matmul(
                            ps, lhsT=t_wpr[:, 0, co * 128:(co + 1) * 128],
                            rhs=y_qc[:, 0, :], start=True, stop=True)
                        oh = ohpool.tile([128, TC], bf16, tag="oh")
                        o_half[co] = oh
                        nc.vector.tensor_copy(out=oh, in_=ps)
                    for co in range(NCT):
                        proj_due.append(lambda co=co: pass1(co))
                attn_half(qc, 1, y_qc, fill, last=(qc == NQC - 1))
            for tcn in range(NQC):
                drain(segment_due[tcn])
            drain(proj_due)
            y_qc = y_qcs[NQC - 1]
            for co in range(NCT):
                ps = mm_ps.tile([128, TC], f32, tag="mm", name="psp2")
                nc.tensor.matmul(
                    ps, lhsT=t_wpr[:, 1, co * 128:(co + 1) * 128],
                    rhs=y_qc[:, 1, :], start=True, stop=True)
                o_sb = rpool.tile([128, TC], bf16, tag="osb")
                nc.vector.tensor_add(out=o_sb, in0=ps, in1=o_half[co])
                nc.sync.dma_start(
                    out=yT_out[co * 128:(co + 1) * 128,
                               (NQC - 1) * TC:NQC * TC],
                    in_=o_sb)

    nc.compile()
    return nc


def _prep_inputs(x, w_qkv, w_proj, freqs_cos, freqs_sin):
    bf = ml_dtypes.bfloat16
    cos = np.asarray(freqs_cos, np.float32)   # [T, 32]
    sin = np.asarray(freqs_sin, np.float32)
    # even/odd-split RoPE: within each head, q/k columns are permuted to
    # [d0,d2,..,d62, d1,d3,..,d63]; patterns are 32-row blocks
    cos_p = np.tile(cos.T, (4, 1)).astype(bf)                  # [128, T]
    sin_p = np.tile(np.concatenate([sin.T, -sin.T], 0), (2, 1)).astype(bf)
    eo = np.concatenate([np.arange(0, HD, 2), np.arange(1, HD, 2)])
    # single tril band mask for all diagonal k-tiles: within a diagonal
    # tile, q_local j (col) >= k_local p (partition) is unmasked
    kp = np.arange(128)[:, None]
    qf = np.arange(128)[None, :]
    m = (qf >= kp).astype(bf)                           # [128, 128]

    x = np.asarray(x, np.float32)
    w_qkv = np.asarray(w_qkv, np.float32)
    w_proj = np.asarray(w_proj, np.float32)
    in_maps = []
    # per-head even/odd column permutation for q and k blocks
    perm = np.concatenate([h * HD + eo for h in range(H)])
    wq_p = w_qkv[:, 0 * C:1 * C][:, perm]
    wk_p = w_qkv[:, 1 * C:2 * C][:, perm]
    for i in range(N_CORES):
        b, t = divmod(i, 4)
        jq = slice(t * JV, (t + 1) * JV)
        wq = wq_p[:, jq]
        wk = wk_p[:, jq]
        wv = w_qkv[:, 2 * C:3 * C][:, jq]
        in_maps.append({
            "xT": np.ascontiguousarray(x[b].T).astype(bf),
            "w_qk": np.concatenate([wq, wk], axis=1).astype(bf),
            "w_v": np.ascontiguousarray(wv).astype(bf),
            "w_pr": np.ascontiguousarray(w_proj[t * JV:(t + 1) * JV, :]).astype(bf),
            "cos_p": cos_p, "sin_p": sin_p, "trimask": m,
        })
    return in_maps


def run(inputs, trace=False):
    from concourse import bass_utils
    if "nc" not in _CACHE:
        _CACHE["nc"] = _build()
    nc = _CACHE["nc"]
    in_maps = _prep_inputs(**inputs)
    res = bass_utils.run_bass_kernel_spmd(
        nc, in_maps, core_ids=list(range(N_CORES)), trace=trace)
    out = np.empty((B, T, C), np.float32)
    for b in range(B):
        acc = res.results[b * 4]["yT"].astype(np.float32)
        for t in range(1, 4):
            acc += res.results[b * 4 + t]["yT"]
        out[b] = acc.T
    return out, res


def kernel(**inputs):
    out, _ = run(inputs, trace=False)
    return out
